# revision 4
# baseline (speedup 1.0000x reference)
"""Trainium2 Bass kernel for nn_CRANModel (CRAN-style memory recurrence).

Strategy
--------
Cache *keys* depend only on token embeddings, so scores/top-8/softmax are
precomputed in one batched phase.  The serial *value* path reduces to

    h_t = tanh(U'_t + A_t @ G[:t]),   G_j = (1^T h_j) @ C'

with U' = [X|R] @ Wh + bh - A@D0 (batched), A = masked top-8 weights on the
first 64 slots, C' = Wv @ Wh_r / B.

The 64-step scan runs TRANSPOSED: pre-activations accumulate as
pz^T [H-chunk x 128, 4 steps * 32 batch] PSUM blocks, so the history matmul
A_t @ G[:t] streams only 32 columns (vs 512 row-major) and a single tanh
per step writes h^T straight into the projection operand hT_sb.  Only the
C'-apply (4 x 512-col f32r matmuls) remains on the per-step chain.

The 262 MB logits projection (the memory roofline) is sharded over vocab
across the 8 cores, computed in bf16 (same PE rate, half the SBUF/DMA),
and interleaved into the scan in 8-step column chunks as PE filler around
the C'-apply, with 4-vocab-tile batched output DMAs.
"""

import sys
import numpy as np
import ml_dtypes

for p in ("/opt/trn_rl_repo", "/root/.axon_site/_ro/trn_rl_repo"):
    if p not in sys.path:
        sys.path.append(p)

# problem dims (hardcoded per contract)
T, B, V, E, H, N, DK, DV = 64, 32, 32000, 512, 512, 512, 256, 512
K = 8
NCORES = 8
TB = T * B                   # 2048 rows
RG = TB // 128               # 16 row groups of 128
VSH = V // NCORES            # 4000 vocab columns per core
VCH = (VSH + 127) // 128     # 32 v-chunks per core (last is ragged: 32 rows)
VLAST = VSH - (VCH - 1) * 128
CN = 8                       # projection column chunks (8 steps = 256 cols)
CW = TB // CN                # 256 cols per chunk
_SCORES_F32R = True          # score path in f32r (1cyc/row); fp32 fallback
_REPEAT = 1
_DEBUG = False


def _round_f32r(a):
    """Round-to-nearest-even to 11 explicit mantissa bits (fp32r)."""
    u = np.ascontiguousarray(a, np.float32).view(np.uint32)
    u = (u + 0x7FF + ((u >> 12) & 1)) & np.uint32(0xFFFFF000)
    return u.view(np.float32)


def _build_program(repeat=1):
    import contextlib
    import concourse.bass as bass
    import concourse.mybir as mybir
    import concourse.tile as tile
    from concourse import bacc
    from concourse.masks import make_identity

    f32 = mybir.dt.float32
    f32r = mybir.dt.float32r
    bf16 = mybir.dt.bfloat16
    ACT = mybir.ActivationFunctionType

    nc = bacc.Bacc("TRN2", debug=False, target_bir_lowering=False)

    # ---------------- DRAM I/O ----------------
    d_tok = nc.dram_tensor("tok", [128, RG], mybir.dt.int32, kind="ExternalInput").ap()
    d_emb = nc.dram_tensor("emb", [V, E], f32, kind="ExternalInput").ap()
    fsc = f32r if _SCORES_F32R else f32
    d_wq = nc.dram_tensor("wq", [E, DK], fsc, kind="ExternalInput").ap()
    d_wk = nc.dram_tensor("wk", [E, DK], fsc, kind="ExternalInput").ap()
    d_k0T = nc.dram_tensor("k0T", [DK, N], fsc, kind="ExternalInput").ap()
    d_wh = nc.dram_tensor("wh", [E + DV, H], f32r, kind="ExternalInput").ap()
    d_wvT = nc.dram_tensor("wvT", [DV, H], f32r, kind="ExternalInput").ap()
    d_v0 = nc.dram_tensor("v0", [N, DV], f32r, kind="ExternalInput").ap()
    d_v0hT = nc.dram_tensor("v0hT", [DV, T], f32r, kind="ExternalInput").ap()
    d_bhb = nc.dram_tensor("bhb", [1, H], f32, kind="ExternalInput").ap()
    d_maskRM = nc.dram_tensor("maskRM", [128, RG, T], f32, kind="ExternalInput").ap()
    d_wout = nc.dram_tensor("woutc", [128, 4, VCH * 128], bf16,
                            kind="ExternalInput").ap()
    d_boutT = nc.dram_tensor("boutc", [128, VCH], f32, kind="ExternalInput").ap()
    d_out = nc.dram_tensor("out", [VSH, TB], bf16, kind="ExternalOutput").ap()
    if _DEBUG:
        d_dbgG = nc.dram_tensor("dbgG", [T, H], f32, kind="ExternalOutput").ap()
        d_dbgA = nc.dram_tensor("dbgA", [T, TB], f32, kind="ExternalOutput").ap()
        d_dbgH = nc.dram_tensor("dbgH", [128, 4, TB], mybir.dt.uint16,
                                kind="ExternalOutput").ap()
        d_dbgW = nc.dram_tensor("dbgW", [128, 4, T], f32,
                                kind="ExternalOutput").ap()

    with tile.TileContext(nc) as tc:
        with contextlib.ExitStack() as stack:
            cst = stack.enter_context(tc.tile_pool(name="cst", bufs=1))

            ident = cst.tile([128, 128], f32)
            make_identity(nc, ident)
            ident_bf = cst.tile([128, 128], bf16)
            nc.vector.tensor_copy(out=ident_bf[:], in_=ident[:])
            tok_sb = cst.tile([128, RG], mybir.dt.int32)
            nc.sync.dma_start(tok_sb[:], d_tok[:])
            boutT_sb = cst.tile([128, VCH], f32)
            nc.sync.dma_start(boutT_sb[:], d_boutT[:])

            # persistent tensors for the scan
            big = stack.enter_context(tc.tile_pool(name="big", bufs=1))
            AT_bf = big.tile([T, TB], bf16)       # masked A^T  [slot, row]
            U_bf = big.tile([128, RG, H], bf16)   # U' rows per 4-step group
            c_sb = big.tile([128, 4, H], f32r)    # C' = Wv @ Wh_r / B
            G_bf = big.tile([T, H], bf16)         # G rows (hbar @ C')
            hT_sb = big.tile([128, 4, TB], bf16)  # h^T for the projection

            for _rep in range(repeat):
                scan_pools = {}

                def sc_pool(name):
                    return scan_pools[name]

                def scan_step(t, fillers=()):
                    j, b = t & 3, t // 4
                    fillers = list(fillers)
                    # per-step psum tile: U' inject (start) + history (stop)
                    pzt = sc_pool("ps_blk").tile([128, 4, B], f32, tag="pzt",
                                                 name="pzt")
                    for m in range(4):
                        msl = slice(m * 128, (m + 1) * 128)
                        nc.tensor.matmul(
                            out=pzt[:, m, :], lhsT=U_bf[:, b, msl],
                            rhs=ident_bf[:, j * B:(j + 1) * B],
                            start=True, stop=(t == 0))
                        if t > 0:
                            # full history A_t @ G[:t] — only 32 cols each
                            nc.tensor.matmul(
                                out=pzt[:, m, :],
                                lhsT=G_bf[0:t, msl],
                                rhs=AT_bf[0:t, t * B:(t + 1) * B],
                                start=False, stop=True)

                    # tanh -> h^T columns (bf16), one instruction
                    nc.scalar.activation(
                        out=hT_sb[:, :, t * B:(t + 1) * B],
                        in_=pzt[:],
                        func=ACT.Tanh)

                    # hbar^T = batch-sum of h^T, written straight into
                    # column t of the one-hot wideG table
                    wideG = sc_pool("wideG")
                    psum_G = sc_pool("psum_G")
                    if t > 0:
                        nc.vector.memset(wideG[:, :, t - 1:t].bitcast(f32), 0.0)
                    with nc.allow_low_precision(
                            reason="hbar rounded to f32r for the PE"):
                        nc.vector.reduce_sum(
                            out=wideG[:, :, t:t + 1],
                            in_=hT_sb[:, :, t * B:(t + 1) * B],
                            axis=mybir.AxisListType.X)

                    # filler PE work issued while tanh+reduce are in flight
                    for f in fillers[:2]:
                        f()

                    # G row t = hbar @ C' into the PSUM G table (rank-1 via
                    # the one-hot column; 4 x 512-col f32r matmuls)
                    for k in range(4):
                        nc.tensor.matmul(
                            out=psum_G[:], lhsT=wideG[:, k, :],
                            rhs=c_sb[:, k, :],
                            start=(t == 0 and k == 0),
                            stop=(t == T - 1 and k == 3),
                            skip_group_check=True)
                    # mirror the updated 32-row block to SBUF (bf16)
                    blk32 = (t // 32) * 32
                    nc.vector.tensor_copy(
                        out=G_bf[blk32:blk32 + 32, 0:256],
                        in_=psum_G[blk32:blk32 + 32, 0:256])
                    nc.scalar.copy(
                        out=G_bf[blk32:blk32 + 32, 256:512],
                        in_=psum_G[blk32:blk32 + 32, 256:512])

                    # filler PE work covering the G-copy latency
                    for f in fillers[2:]:
                        f()

                # =================== PHASE 0 ===================
                with contextlib.ExitStack() as ph0:
                    w0 = ph0.enter_context(tc.tile_pool(name="w0", bufs=1))
                    xt_p = ph0.enter_context(tc.tile_pool(name="xt", bufs=1))
                    p0 = ph0.enter_context(tc.tile_pool(name="p0", bufs=1))
                    pp = ph0.enter_context(tc.tile_pool(name="pp", bufs=2))
                    px = ph0.enter_context(tc.tile_pool(name="px", bufs=6))
                    wp = ph0.enter_context(tc.tile_pool(name="wp", bufs=2))
                    qp = ph0.enter_context(tc.tile_pool(name="qp", bufs=1))
                    ps_mm = ph0.enter_context(
                        tc.tile_pool(name="ps_mm", bufs=4, space="PSUM"))
                    ps_tr = ph0.enter_context(
                        tc.tile_pool(name="ps_tr", bufs=4, space="PSUM"))

                    wq_sb = w0.tile([128, 4, DK], fsc)
                    nc.sync.dma_start(
                        wq_sb[:], d_wq.rearrange("(c p) m -> p c m", p=128))
                    wk_sb = w0.tile([128, 4, DK], fsc)
                    nc.sync.dma_start(
                        wk_sb[:], d_wk.rearrange("(c p) m -> p c m", p=128))
                    k0T_sb = w0.tile([128, 2, N], fsc)
                    nc.sync.dma_start(
                        k0T_sb[:], d_k0T.rearrange("(c p) m -> p c m", p=128))
                    # chunked loads: keep individual transfers ~0.7 us so
                    # the pass-A gather DMAs are never stuck behind them
                    wh_sb = w0.tile([128, 8, H], f32r)
                    whr_ap = d_wh.rearrange("(c p) m -> p c m", p=128)
                    for c8 in range(8):
                        nc.sync.dma_start(wh_sb[:, c8, :], whr_ap[:, c8, :])
                    wvT_sb = w0.tile([128, 4, H], f32r)
                    wvr_ap = d_wvT.rearrange("(c p) m -> p c m", p=128)
                    for c4 in range(4):
                        nc.sync.dma_start(wvT_sb[:, c4, :], wvr_ap[:, c4, :])
                    v0_sb = w0.tile([128, 4, DV], f32r)
                    v0r_ap = d_v0.rearrange("(c p) m -> p c m", p=128)
                    for c4 in range(4):
                        nc.sync.dma_start(v0_sb[:, c4, :], v0r_ap[:, c4, :])
                    v0hT_sb = w0.tile([128, 4, T], f32r)
                    nc.sync.dma_start(
                        v0hT_sb[:], d_v0hT.rearrange("(c p) m -> p c m", p=128))
                    bhb_sb = w0.tile([1, H], f32)
                    nc.sync.dma_start(bhb_sb[:], d_bhb[:])
                    bhr_sb = w0.tile([1, H], f32r)
                    nc.vector.tensor_copy(out=bhr_sb[:], in_=bhb_sb[:])
                    ones32 = w0.tile([1, 128], f32)
                    nc.vector.memset(ones32[:], 1.0)
                    onesr = w0.tile([1, 128], f32r)
                    nc.vector.tensor_copy(out=onesr[:], in_=ones32[:])
                    maskRM_sb = w0.tile([128, RG, T], f32)
                    nc.sync.dma_start(maskRM_sb[:], d_maskRM[:])
                    negD0 = w0.tile([T, H], bf16)

                    xT_sb = xt_p.tile([128, 4, TB], fsc)
                    xbT_sb = xt_p.tile([128, 4, T], fsc)
                    knT_sb = xt_p.tile([128, 2, T], fsc)

                    # --- pass A: gather X = emb[tok], transpose into xT ---
                    for g in range(RG):
                        xg = px.tile([128, E], f32, tag="xg")
                        nc.gpsimd.indirect_dma_start(
                            out=xg[:], out_offset=None, in_=d_emb[:],
                            in_offset=bass.IndirectOffsetOnAxis(
                                ap=tok_sb[:, g:g + 1], axis=0),
                        )
                        for e in range(4):
                            ptr = ps_tr.tile([128, 128], f32, tag="ptr")
                            nc.tensor.transpose(
                                out=ptr[:], in_=xg[:, e * 128:(e + 1) * 128],
                                identity=ident[:])
                            if e % 2 == 0:
                                nc.scalar.copy(
                                    out=xT_sb[:, e, g * 128:(g + 1) * 128],
                                    in_=ptr[:])
                            else:
                                nc.vector.tensor_copy(
                                    out=xT_sb[:, e, g * 128:(g + 1) * 128],
                                    in_=ptr[:])

                    # --- Xbar^T (batch sums; 1/B folded into Knew evict) ---
                    with nc.allow_low_precision(
                            reason="batch-mean rounded to f32r for the PE; "
                                   "accumulator is fp32"):
                        for e in range(4):
                            nc.vector.reduce_sum(
                                out=xbT_sb[:, e, :],
                                in_=xT_sb[:, e, :].rearrange(
                                    "p (t b) -> p t b", b=B),
                                axis=mybir.AxisListType.X)

                    # --- Knew^T = Wk^T Xbar^T / B ---
                    for m2 in range(2):
                        pk = ps_mm.tile([128, 512], f32, tag="pmm")
                        for e in range(4):
                            nc.tensor.matmul(
                                out=pk[:, 0:T],
                                lhsT=wk_sb[:, e, m2 * 128:(m2 + 1) * 128],
                                rhs=xbT_sb[:, e, :],
                                start=(e == 0), stop=(e == 3))
                        nc.scalar.activation(
                            out=knT_sb[:, m2, :], in_=pk[:, 0:T],
                            func=ACT.Copy, scale=float(1.0 / B))

                    # --- C' = Wv @ Wh_r / B ;  negD0 = -values0[:64] @ Wh_r ---
                    for m4 in range(4):
                        pc = ps_mm.tile([128, H], f32, tag="pmm")
                        for d4 in range(4):
                            nc.tensor.matmul(
                                out=pc[:],
                                lhsT=wvT_sb[:, d4, m4 * 128:(m4 + 1) * 128],
                                rhs=wh_sb[:, 4 + d4, :], start=(d4 == 0),
                                stop=(d4 == 3))
                        nc.scalar.activation(out=c_sb[:, m4, :], in_=pc[:],
                                             func=ACT.Copy, scale=float(1.0 / B))
                    pd = ps_mm.tile([128, H], f32, tag="pmm")
                    for d4 in range(4):
                        nc.tensor.matmul(
                            out=pd[0:T, :], lhsT=v0hT_sb[:, d4, :],
                            rhs=wh_sb[:, 4 + d4, :], start=(d4 == 0),
                            stop=(d4 == 3))
                    nc.scalar.activation(out=negD0[:], in_=pd[0:T, :],
                                         func=ACT.Copy, scale=-1.0)

                    # --- pass B: per quad of row-groups (N=512 matmuls),
                    # software-pipelined: scores/top-8 of q4 run while the
                    # transposes/R/U of q4-1 occupy the PE; scan steps 0-15
                    # are interleaved so their serial chain hides here too.
                    def emit_front(q4):
                        qsl = slice(q4 * 512, (q4 + 1) * 512)

                        qT4 = pp.tile([128, 2, 512], fsc, tag="qT4")
                        for m2 in range(2):
                            pq = ps_mm.tile([128, 512], f32, tag="pmm")
                            for e in range(4):
                                nc.tensor.matmul(
                                    out=pq[:],
                                    lhsT=wq_sb[:, e, m2 * 128:(m2 + 1) * 128],
                                    rhs=xT_sb[:, e, qsl],
                                    start=(e == 0), stop=(e == 3))
                            nc.scalar.activation(
                                out=qT4[:, m2, :], in_=pq[:],
                                func=ACT.Copy, scale=float(1.0 / np.sqrt(DK)))

                        wgs = []
                        for gl in range(4):
                            g = q4 * 4 + gl
                            lsl = slice(gl * 128, (gl + 1) * 128)

                            s_g = p0.tile([128, N], f32, tag=f"sg{gl}")
                            ps_s = ps_mm.tile([128, N], f32, tag="pmm")
                            for k2 in range(2):
                                nc.tensor.matmul(
                                    out=ps_s[:], lhsT=qT4[:, k2, lsl],
                                    rhs=k0T_sb[:, k2, :],
                                    start=(k2 == 0), stop=(k2 == 1))
                            nc.scalar.copy(out=s_g[:], in_=ps_s[:])
                            ps_n = ps_mm.tile([128, N], f32, tag="pmm")
                            for k2 in range(2):
                                nc.tensor.matmul(
                                    out=ps_n[:, 0:T], lhsT=qT4[:, k2, lsl],
                                    rhs=knT_sb[:, k2, :],
                                    start=(k2 == 0), stop=(k2 == 1))
                            nc.vector.copy_predicated(
                                out=s_g[:, 0:T],
                                mask=maskRM_sb[:, g, :].bitcast(mybir.dt.uint32),
                                data=ps_n[:, 0:T])

                            # top-8 softmax, normalizer folded into exp bias:
                            # w = (s >= thr) * exp(s - mx - ln z)
                            mx = p0.tile([128, 8], f32, tag=f"mx{gl}")
                            nc.vector.max(out=mx[:], in_=s_g[:])
                            negm1 = p0.tile([128, 1], f32, tag=f"nm{gl}")
                            nc.vector.tensor_scalar_mul(negm1[:], mx[:, 0:1],
                                                        -1.0)
                            emx = p0.tile([128, 8], f32, tag=f"em{gl}")
                            nc.scalar.activation(out=emx[:], in_=mx[:],
                                                 func=ACT.Exp,
                                                 bias=negm1[:, 0:1])
                            zrow = p0.tile([128, 1], f32, tag=f"zr{gl}")
                            nc.vector.reduce_sum(out=zrow[:], in_=emx[:],
                                                 axis=mybir.AxisListType.X)
                            winv = p0.tile([128, 1], f32, tag=f"wi{gl}")
                            nc.vector.reciprocal(out=winv[:], in_=zrow[:])
                            eb = p0.tile([128, N], f32, tag=f"eb{gl}")
                            nc.scalar.activation(out=eb[:], in_=s_g[:],
                                                 func=ACT.Exp,
                                                 bias=negm1[:, 0:1])
                            w_g = wp.tile([128, N], f32, tag=f"wg{gl}")
                            nc.vector.scalar_tensor_tensor(
                                out=w_g[:], in0=s_g[:], scalar=mx[:, 7:8],
                                in1=eb[:], op0=mybir.AluOpType.is_ge,
                                op1=mybir.AluOpType.mult)
                            nc.vector.tensor_scalar_mul(w_g[:], w_g[:],
                                                        winv[:, 0:1])
                            am = wp.tile([128, T], f32, tag=f"am{gl}")
                            nc.gpsimd.tensor_mul(am[:], w_g[:, 0:T],
                                                 maskRM_sb[:, g, :])
                            wgs.append((w_g, am))
                        return wgs

                    def emit_back(q4, wgs):
                        # transposes into A^T / Wfull^T
                        wfT4 = qp.tile([128, 4, 512], f32r, tag="wfT4")
                        for gl in range(4):
                            g = q4 * 4 + gl
                            gsl = slice(g * 128, (g + 1) * 128)
                            lsl = slice(gl * 128, (gl + 1) * 128)
                            w_g, am = wgs[gl]

                            pat = ps_tr.tile([128, 128], f32, tag="ptr")
                            nc.tensor.transpose(out=pat[0:T, :], in_=am[:],
                                                identity=ident[:])
                            nc.vector.tensor_copy(out=AT_bf[:, gsl],
                                                  in_=pat[0:T, :])

                            for s4 in range(4):
                                ptr = ps_tr.tile([128, 128], f32, tag="ptr")
                                nc.tensor.transpose(
                                    out=ptr[:],
                                    in_=w_g[:, s4 * 128:(s4 + 1) * 128],
                                    identity=ident[:])
                                if s4 % 2 == 0:
                                    nc.scalar.copy(out=wfT4[:, s4, lsl],
                                                   in_=ptr[:])
                                else:
                                    nc.vector.tensor_copy(out=wfT4[:, s4, lsl],
                                                          in_=ptr[:])

                        # R^T quad = values0^T @ Wfull^T   (f32r, N=512)
                        rT4 = qp.tile([128, 4, 512], f32r, tag="rT4")
                        for m4 in range(4):
                            pr = ps_mm.tile([128, 512], f32, tag="pmm")
                            for s4 in range(4):
                                nc.tensor.matmul(
                                    out=pr[:],
                                    lhsT=v0_sb[:, s4, m4 * 128:(m4 + 1) * 128],
                                    rhs=wfT4[:, s4, :],
                                    start=(s4 == 0), stop=(s4 == 3))
                            if m4 % 2 == 0:
                                nc.vector.tensor_copy(out=rT4[:, m4, :],
                                                      in_=pr[:])
                            else:
                                nc.scalar.copy(out=rT4[:, m4, :], in_=pr[:])

                        # U' rows = [X|R] @ Wh + bh + A@(-D0)   -> bf16
                        for gl in range(4):
                            g = q4 * 4 + gl
                            gsl = slice(g * 128, (g + 1) * 128)
                            lsl = slice(gl * 128, (gl + 1) * 128)
                            pu = ps_mm.tile([128, H], f32, tag="pmm")
                            for e in range(4):
                                nc.tensor.matmul(
                                    out=pu[:], lhsT=xT_sb[:, e, gsl],
                                    rhs=wh_sb[:, e, :], start=(e == 0),
                                    stop=False)
                            for d4 in range(4):
                                nc.tensor.matmul(
                                    out=pu[:], lhsT=rT4[:, d4, lsl],
                                    rhs=wh_sb[:, 4 + d4, :], start=False,
                                    stop=False)
                            nc.tensor.matmul(
                                out=pu[:], lhsT=onesr[:], rhs=bhr_sb[:],
                                start=False, stop=False)
                            nc.tensor.matmul(
                                out=pu[:], lhsT=AT_bf[:, gsl], rhs=negD0[:],
                                start=False, stop=True)
                            if gl % 2 == 0:
                                nc.vector.tensor_copy(out=U_bf[:, g, :],
                                                      in_=pu[:])
                            else:
                                nc.scalar.copy(out=U_bf[:, g, :], in_=pu[:])

                    fronts = {0: emit_front(0)}
                    for q4 in range(4):
                        if q4 + 1 < 4:
                            fronts[q4 + 1] = emit_front(q4 + 1)
                        emit_back(q4, fronts.pop(q4))

                # ===== scan steps 16-63 with interleaved projection =====
                with contextlib.ExitStack() as ph1:
                    wop = ph1.enter_context(tc.tile_pool(name="wop", bufs=1))
                    ob_p = ph1.enter_context(tc.tile_pool(name="ob", bufs=2))
                    scan_pools["sc"] = ph1.enter_context(
                        tc.tile_pool(name="sc", bufs=3))
                    scan_pools["ps_blk"] = ph1.enter_context(
                        tc.tile_pool(name="ps_blk", bufs=3, space="PSUM"))
                    ps_G_pool = ph1.enter_context(
                        tc.tile_pool(name="ps_G", bufs=1, space="PSUM"))
                    ps_o = ph1.enter_context(
                        tc.tile_pool(name="ps_o", bufs=3, space="PSUM"))
                    # one-hot hbar columns (col t = hbar^T at step t); the
                    # C'-apply streams it as lhsT so row t of the PSUM G
                    # table receives hbar @ C' without unaligned writes
                    wideG = wop.tile([128, 4, T], f32r)
                    nc.vector.memset(wideG[:].bitcast(f32), 0.0)
                    psum_G = ps_G_pool.tile([T, H], f32)
                    scan_pools["wideG"] = wideG
                    scan_pools["psum_G"] = psum_G

                    # split load: first-half vocab tiles arrive sooner so
                    # early projection units aren't blocked on the full 4 MB
                    wout_a = wop.tile([128, 4, VCH * 64], bf16)
                    for q in range(4):
                        nc.sync.dma_start(
                            wout_a[:, q, :], d_wout[:, q, 0:VCH * 64])
                    wout_b = wop.tile([128, 4, VCH * 64], bf16)
                    for q in range(4):
                        nc.sync.dma_start(
                            wout_b[:, q, :], d_wout[:, q, VCH * 64:])

                    # projection: unit = (cN column-chunk, vc) -> 4 matmuls
                    ob_tiles = {}

                    def proj_unit(cn, vc):
                        vsz = 128 if vc < VCH - 1 else VLAST
                        csl = slice(cn * CW, (cn + 1) * CW)
                        po = ps_o.tile([128, CW], f32, tag="po")
                        wsb = wout_a if vc < VCH // 2 else wout_b
                        vr = vc % (VCH // 2)
                        for hc in range(4):
                            nc.tensor.matmul(
                                out=po[:],
                                lhsT=wsb[:, hc, vr * 128:(vr + 1) * 128],
                                rhs=hT_sb[:, hc, csl],
                                start=(hc == 0), stop=(hc == 3))
                        if vc % 4 == 0:
                            ob_tiles[cn] = ob_p.tile([128, 4, CW], bf16,
                                                     tag="ob", name="ob")
                        ob = ob_tiles[cn]
                        if vc % 2 == 0:
                            nc.scalar.activation(
                                out=ob[0:vsz, vc % 4, :], in_=po[0:vsz, :],
                                func=ACT.Identity,
                                bias=boutT_sb[0:vsz, vc:vc + 1])
                        else:
                            nc.vector.tensor_scalar_add(
                                ob[0:vsz, vc % 4, :], po[0:vsz, :],
                                boutT_sb[0:vsz, vc:vc + 1])
                        if vc % 4 == 3:
                            v0c = vc - 3
                            if vc < VCH - 1:
                                nc.sync.dma_start(
                                    d_out[v0c * 128:(vc + 1) * 128, csl]
                                    .rearrange("(v p) c -> p v c", p=128),
                                    ob[:])
                            else:
                                nc.sync.dma_start(
                                    d_out[v0c * 128:vc * 128, csl]
                                    .rearrange("(v p) c -> p v c", p=128),
                                    ob[:, 0:3, :])
                                nc.sync.dma_start(
                                    d_out[vc * 128:vc * 128 + VLAST, csl],
                                    ob[0:VLAST, 3, :])

                    # proj schedule over steps 16..63 + tail; chunk cn ready
                    # after step 8*(cn+1)-1
                    proj_sched = {t: [] for t in range(8, T)}
                    for cn in range(CN):
                        t0 = 8 * (cn + 1)
                        for vc in range(VCH):
                            t_em = min(t0 + (vc // 4), T - 1)
                            proj_sched[t_em].append((cn, vc))
                    carry = []
                    for t in range(8, T):
                        avail = proj_sched[t] + carry
                        take = min(len(avail), 7) if t < T - 1 else len(avail)
                        proj_sched[t], carry = avail[:take], avail[take:]

                    for t in range(0, T):
                        units = [
                            (lambda cn=cn, vc=vc: proj_unit(cn, vc))
                            for cn, vc in proj_sched.get(t, [])]
                        scan_step(t, units)

                    if _DEBUG:
                        dbgG = wop.tile([T, H], f32)
                        nc.vector.tensor_copy(out=dbgG[:], in_=G_bf[:])
                        nc.sync.dma_start(d_dbgG[:], dbgG[:])
                        dbgA = wop.tile([T, TB], f32)
                        nc.vector.tensor_copy(out=dbgA[:], in_=AT_bf[:])
                        nc.sync.dma_start(d_dbgA[:], dbgA[:])
                        nc.sync.dma_start(
                            d_dbgH[:], hT_sb[:].bitcast(mybir.dt.uint16))
                        dbgW = wop.tile([128, 4, T], f32)
                        nc.vector.tensor_copy(out=dbgW[:],
                                              in_=scan_pools["wideG"][:])
                        nc.sync.dma_start(d_dbgW[:], dbgW[:])

    nc.compile()
    return nc


_CACHE = {}


def _get_program():
    key = ("nc", _REPEAT, _DEBUG)
    if key not in _CACHE:
        _CACHE[key] = _build_program(repeat=_REPEAT)
    return _CACHE[key]


def _host_prep(tokens, emb, Wq, Wk, Wv, Wh, bh, Wout, bout, keys0, values0):
    tok = np.ascontiguousarray(
        np.asarray(tokens, np.int64).reshape(TB).astype(np.int32))
    tok_cm = np.zeros((128, RG), np.int32)
    for g in range(RG):
        tok_cm[:, g] = tok[g * 128:(g + 1) * 128]

    t_of_row = np.repeat(np.arange(T), B)                      # [TB]
    maskRM = (np.arange(T)[None, :] < t_of_row[:, None]).astype(np.float32)
    maskRM_cm = np.zeros((128, RG, T), np.float32)
    for g in range(RG):
        maskRM_cm[:, g, :] = maskRM[g * 128:(g + 1) * 128]

    fsc = _round_f32r if _SCORES_F32R else np.ascontiguousarray
    base = {
        "tok": tok_cm,
        "emb": np.ascontiguousarray(np.asarray(emb, np.float32)),
        "wq": fsc(np.asarray(Wq, np.float32)),
        "wk": fsc(np.asarray(Wk, np.float32)),
        "k0T": fsc(np.asarray(keys0, np.float32).T),
        "wh": _round_f32r(np.asarray(Wh, np.float32)),
        "wvT": _round_f32r(np.asarray(Wv, np.float32).T),
        "v0": _round_f32r(np.asarray(values0, np.float32)),
        "v0hT": _round_f32r(np.asarray(values0, np.float32)[:T].T),
        "bhb": np.ascontiguousarray(
            np.asarray(bh, np.float32).reshape(1, H)),
        "maskRM": maskRM_cm,
    }

    Wout = np.asarray(Wout, np.float32)
    bout = np.asarray(bout, np.float32)
    in_maps = []
    for c in range(NCORES):
        wsh = Wout[:, c * VSH:(c + 1) * VSH]           # [H, VSH]
        wt = np.zeros((128, 4, VCH * 128), np.float32)
        for hc in range(4):
            wt[:, hc, :VSH] = wsh[hc * 128:(hc + 1) * 128, :]
        bt = np.zeros((128, VCH), np.float32)
        bsh = bout[c * VSH:(c + 1) * VSH]
        for vc in range(VCH):
            vsz = 128 if vc < VCH - 1 else VLAST
            bt[:vsz, vc] = bsh[vc * 128:vc * 128 + vsz]
        in_maps.append({**base,
                        "woutc": wt.astype(ml_dtypes.bfloat16),
                        "boutc": bt})
    return in_maps


def run_on_device(in_maps, trace=False):
    from concourse import bass_utils
    nc = _get_program()
    return bass_utils.run_bass_kernel_spmd(
        nc, in_maps, core_ids=list(range(NCORES)), trace=trace)


def kernel(tokens, emb, Wq, Wk, Wv, Wh, bh, Wout, bout, keys0, values0, k):
    assert int(k) == K
    in_maps = _host_prep(tokens, emb, Wq, Wk, Wv, Wh, bh, Wout, bout,
                         keys0, values0)
    res = run_on_device(in_maps)
    parts = [np.asarray(res.results[c]["out"], dtype=np.float32)
             for c in range(NCORES)]                             # each [VSH, TB]
    logitsT = np.concatenate(parts, axis=0)                      # [V, TB]
    return np.ascontiguousarray(logitsT.T).reshape(T, B, V)

